# revision 24
# baseline (speedup 1.0000x reference)
"""DenseGATConv Trainium2 kernel.

Problem: B=4, N=2048, F=128, H=4, C=32.
  xh = (x @ W).reshape(B, N, H, C)
  a_src[b,j,h] = <xh[b,j,h,:], att_src[h,:]>;  a_dst likewise
  alpha[b,i,j,h] = softmax_j( mask(adj+selfloop, leaky_relu(a_src[j]+a_dst[i])) )
  out[b,i,:] = concat_h( sum_j alpha[b,i,j,h] * xh[b,j,h,:] ) + bias

Sharding: 8 cores = (4 batches) x (2 halves of dst rows i). Each core owns a
[1024, 2048, 4] alpha slab and [1024, 128] out slab. SPMD: one program, per-core
input slices. Host fixes the self-loop diagonal in each core's int32 adj slice.

Per-core device pipeline, per (i-tile of 128 rows, head):
  PE    : pre[i,j] = a_dst[i] + a_src[j] via K=2 matmul into PSUM (f32r full-rate)
  ACT   : lk = LeakyRelu(pre, alpha=0.2)   (PSUM -> SBUF)
  ACT   : ex = Exp(lk)
  DVE   : pm = ex * mask, s = row-sum(pm)  (single tensor_tensor_reduce)
  DVE   : r = 1/s
  GPSIMD: alpha[:, :, h] = pm * r   written h-interleaved so the HBM alpha
          write is one fully contiguous 4 MiB DMA per i-tile
  PE    : 16x 128x128 transposes of pm (bf16) + 16 accumulating bf16 matmuls
          against xh -> out_acc;   out = out_acc * r + bias  (linearity lets the
          1/s scale move after the matmul)
"""

import os
import sys
import threading

import numpy as np

if "/opt/trn_rl_repo" not in sys.path:
    sys.path.insert(0, "/opt/trn_rl_repo")

import concourse.bass as bass
import concourse.mybir as mybir
from concourse.masks import make_identity
from concourse.tile import TileContext
from concourse.tile_rust import add_dep_helper

F32 = mybir.dt.float32
F32R = mybir.dt.float32r
BF16 = mybir.dt.bfloat16
I32 = mybir.dt.int32
AOT = mybir.AluOpType
AFT = mybir.ActivationFunctionType

B, N, F, H, C = 4, 2048, 128, 4, 32
HC = H * C
NEG = 0.2
P = 128
NCORES = 8
NI = N // 2  # dst rows per core


def _cdiv(a, b):
    return (a + b - 1) // b


def _split_multi_waits(nc, maxw=1):
    """This toolchain's walrus rejects instructions carrying more than one
    sync-wait (per-struct limits). Hoist extra waits into single-wait
    EventSemaphore instructions on the same engine queue."""
    n = 0
    for fn in nc.m.functions:
        for blk in fn.blocks:
            out = []
            for inst in blk.instructions:
                si = inst.sync_info
                if si is not None and len(si.on_wait) > maxw:
                    for w in list(si.on_wait[maxw:]):
                        n += 1
                        out.append(
                            mybir.InstEventSemaphore(
                                name=f"wsplit-{n}",
                                engine=inst.engine,
                                ins=[],
                                outs=[],
                                sync_info=mybir.SyncInfo(
                                    on_wait=[w], on_update=[]
                                ),
                            )
                        )
                    si.on_wait = list(si.on_wait[:maxw])
                out.append(inst)
            blk.instructions = out
    return n


def build_gat_nc(
    ni=NI,
    nj=N,
    pm_bf16=True,
    int_mask=True,
    scale_engine="gpsimd",
    split_waits=True,
):
    """Build the SPMD Bass program for one core's slab."""
    assert ni % P == 0 and nj % P == 0
    TI, TJ = ni // P, nj // P
    PM_DT = BF16 if pm_bf16 else F32
    # pmT staging: chunks of j-tiles per PSUM transpose tile (1 bank each)
    G = min(8 if pm_bf16 else 4, TJ)
    assert TJ % G == 0
    nc = bass.Bass(trn_type="TRN2", target_bir_lowering=False)

    xb = nc.dram_tensor("xb", [nj, F], F32, kind="ExternalInput")
    xd = nc.dram_tensor("xd", [ni, F], F32, kind="ExternalInput")
    adjs = nc.dram_tensor("adjs", [ni, nj], I32, kind="ExternalInput")
    win = nc.dram_tensor("win", [F, HC], F32, kind="ExternalInput")
    attbd = nc.dram_tensor("attbd", [HC, H], F32, kind="ExternalInput")
    attdb = nc.dram_tensor("attdb", [P, HC], F32, kind="ExternalInput")
    biasb = nc.dram_tensor("biasb", [P, HC], F32, kind="ExternalInput")
    alpha_o = nc.dram_tensor("alpha_o", [ni, nj, H], F32, kind="ExternalOutput")
    out_o = nc.dram_tensor("out_o", [ni, HC], F32, kind="ExternalOutput")

    with TileContext(nc) as tc:
        with (
            tc.tile_pool(name="const", bufs=1) as const_pool,
            tc.tile_pool(name="work", bufs=2) as work,
            tc.tile_pool(name="opool", bufs=2) as opool,
            tc.tile_pool(name="maskp", bufs=2) as maskp,
            tc.tile_pool(name="pmtp", bufs=3) as pmtp,
            tc.tile_pool(name="small", bufs=2) as small,
            tc.tile_pool(name="ps_tp", bufs=4, space="PSUM") as ps_tp,
            tc.tile_pool(name="ps_agg", bufs=2, space="PSUM") as ps_agg,
        ):
            # ---------------- constants ----------------
            w_sb = const_pool.tile([F, HC], F32)
            nc.sync.dma_start(out=w_sb, in_=win[:])
            attbd_sb = const_pool.tile([HC, H], F32)
            nc.sync.dma_start(out=attbd_sb, in_=attbd[:])
            attdb_sb = const_pool.tile([P, HC], F32)
            nc.sync.dma_start(out=attdb_sb, in_=attdb[:])
            biasb_sb = const_pool.tile([P, HC], F32)
            nc.sync.dma_start(out=biasb_sb, in_=biasb[:])
            ident_f = const_pool.tile([P, P], F32)
            make_identity(nc, ident_f)
            if pm_bf16:
                ident_b = const_pool.tile([P, P], BF16)
                make_identity(nc, ident_b)
            else:
                ident_b = ident_f
            xh_bf = const_pool.tile([P, TJ, HC], PM_DT)  # xh[j, h*C+c], j-tiled
            asrc_b = const_pool.tile([P, H * nj], F32)  # a_src bcast to all parts
            adst = const_pool.tile([P, TI, H], F32)  # a_dst columns per i-tile
            adst2 = const_pool.tile([P, TI, H], F32)  # 0.2 * a_dst
            ones_sb = const_pool.tile([1, P], F32)
            nc.vector.memset(ones_sb, 1.0)

            # ---------------- setup: xh, a_src, a_dst ----------------
            with tc.tile_pool(name="setup", bufs=1) as setup_pool:
                xb_sb = setup_pool.tile([P, TJ, F], F32)
                nc.sync.dma_start(
                    out=xb_sb, in_=xb.rearrange("(t p) f -> p t f", p=P)
                )
                xd_sb = setup_pool.tile([P, TI, F], F32)
                nc.sync.dma_start(
                    out=xd_sb, in_=xd.rearrange("(t p) f -> p t f", p=P)
                )

                # Wait-ladder: walrus allows only ONE sync-wait on an
                # (unsplit) f32 Matmult. Warm the PE vector clock one
                # semaphore at a time with dummy transposes into a shared
                # scratch bank (WAW on the tile serializes them) so every
                # real f32 matmul below carries at most one wait.
                warm = ps_tp.tile([P, P], F32, tag="warm", bufs=1, name="warm")
                t_id = nc.tensor.transpose(warm, ident_f, ident_f)
                if pm_bf16:
                    wb = warm.bitcast(BF16)[:, :P]
                    t_id = nc.tensor.transpose(wb, ident_b, ident_b)
                t_w = nc.tensor.transpose(warm, w_sb, ident_f)
                t_att = nc.tensor.transpose(warm[:H, :], attbd_sb, ident_f)
                t_xd = nc.tensor.transpose(warm, xd_sb[:, 0, :], ident_f)

                xbT = setup_pool.tile([P, TJ, P], F32)  # x[b]^T  [F, j]
                xdT = setup_pool.tile([P, TI, P], F32)
                for t in range(TJ):
                    pst = ps_tp.tile([P, P], F32, tag="tp", name="pst")
                    mm = nc.tensor.transpose(pst, xb_sb[:, t, :], ident_f)
                    if t == 0:
                        add_dep_helper(mm.ins, t_xd.ins, False, "pe warm order")
                    nc.scalar.copy(out=xbT[:, t, :], in_=pst)
                for t in range(TI):
                    pst = ps_tp.tile([P, P], F32, tag="tp", name="pst")
                    mm = nc.tensor.transpose(pst, xd_sb[:, t, :], ident_f)
                    if t == 0:
                        add_dep_helper(mm.ins, t_xd.ins, False, "pe warm order")
                    nc.scalar.copy(out=xdT[:, t, :], in_=pst)

                xhT = setup_pool.tile([P, TJ, P], F32)  # xh^T [hc, j]
                junk = setup_pool.tile([P, C], F32)
                for t in range(TJ):
                    psx = ps_tp.tile([P, P], F32, tag="tp", name="psx")
                    nc.tensor.matmul(
                        psx, lhsT=w_sb, rhs=xbT[:, t, :], start=True, stop=True
                    )
                    nc.scalar.copy(out=xhT[:, t, :], in_=psx)
                    psn = ps_tp.tile([P, P], F32, tag="tp", name="psn")
                    nc.tensor.matmul(
                        psn, lhsT=xbT[:, t, :], rhs=w_sb, start=True, stop=True
                    )
                    nc.scalar.copy(out=xh_bf[:, t, :], in_=psn)
                for t in range(TI):
                    # xh of the dst rows, natural [i, hc]; reduce vs att_dst
                    psd = ps_tp.tile([P, P], F32, tag="tp", name="psd")
                    nc.tensor.matmul(
                        psd, lhsT=xdT[:, t, :], rhs=w_sb, start=True, stop=True
                    )
                    xhd_t = setup_pool.tile([P, HC], F32, bufs=2, tag="xhd_t",
                                            name="xhd_t")
                    nc.scalar.copy(out=xhd_t, in_=psd)
                    for h in range(H):
                        nc.vector.scalar_tensor_tensor(
                            out=junk,
                            in0=xhd_t[:, h * C : (h + 1) * C],
                            scalar=0.0,
                            in1=attdb_sb[:, h * C : (h + 1) * C],
                            op0=AOT.bypass,
                            op1=AOT.mult,
                            accum_out=adst[:, t, h : h + 1],
                        )

                # a_srcT rows -> SBUF row -> partition-broadcast to asrc_b
                xhT_flat = xhT.rearrange("p t f -> p (t f)")
                for h in range(H):
                    for q in range(_cdiv(nj, 512)):
                        w = min(512, nj - q * 512)
                        psa = ps_agg.tile([1, 512], F32, tag="agg", name="psa")
                        nc.tensor.matmul(
                            psa[:, :w],
                            lhsT=attbd_sb[:, h : h + 1],
                            rhs=xhT_flat[:, q * 512 : q * 512 + w],
                            start=True,
                            stop=True,
                        )
                        arow = setup_pool.tile([1, 512], F32, tag="arow",
                                               bufs=2, name="arow")
                        nc.scalar.copy(out=arow[:, :w], in_=psa[:, :w])
                        # broadcast to all 128 partitions: ones-column x arow
                        psb = ps_tp.tile([P, 512], F32, tag="tp", name="psb")
                        nc.tensor.matmul(
                            psb[:, :w],
                            lhsT=ones_sb,
                            rhs=arow[0:1, :w],
                            start=True,
                            stop=True,
                        )
                        nc.scalar.copy(
                            out=asrc_b[:, h * nj + q * 512 : h * nj + q * 512 + w],
                            in_=psb[:, :w],
                        )

            nc.scalar.mul(
                out=adst2.rearrange("p a b -> p (a b)"),
                in_=adst.rearrange("p a b -> p (a b)"),
                mul=NEG,
            )

            # ---------------- main loop ----------------
            for t in range(TI):
                adj_t = maskp.tile([P, nj], I32, tag="adj", name="adj_t")
                nc.sync.dma_start(out=adj_t, in_=adjs[t * P : (t + 1) * P, :])
                if int_mask and not pm_bf16:
                    mask_in = adj_t
                else:
                    mask01 = maskp.tile([P, nj], PM_DT, tag="m01", name="mask01")
                    nc.gpsimd.tensor_copy(out=mask01, in_=adj_t)
                    mask_in = mask01

                o_t = opool.tile([P, nj, H], F32, name="o_t")
                s_t = small.tile([P, H], F32, tag="s", name="s_t")
                r_t = small.tile([P, H], F32, tag="r", name="r_t")
                out_sb = small.tile([P, HC], F32, tag="osb", name="out_sb")

                for h in range(H):
                    # exp(leaky_relu(u)) == max(exp(u), exp(0.2*u)) since both
                    # branches are monotone and cross at u=0.
                    e1 = work.tile([P, nj], PM_DT, tag="e1", bufs=2, name="e1")
                    nc.scalar.activation(
                        out=e1,
                        in_=asrc_b[:, h * nj : (h + 1) * nj],
                        func=AFT.Exp,
                        bias=adst[:, t, h : h + 1],
                        scale=1.0,
                    )
                    e2 = work.tile([P, nj], PM_DT, tag="e2", bufs=2, name="e2")
                    nc.scalar.activation(
                        out=e2,
                        in_=asrc_b[:, h * nj : (h + 1) * nj],
                        func=AFT.Exp,
                        bias=adst2[:, t, h : h + 1],
                        scale=NEG,
                    )
                    emx = work.tile([P, nj], PM_DT, tag="pm", bufs=3, name="emx")
                    nc.vector.scalar_tensor_tensor(
                        out=emx,
                        in0=e1,
                        scalar=0.0,
                        in1=e2,
                        op0=AOT.bypass,
                        op1=AOT.max,
                    )
                    pm = work.tile([P, nj], PM_DT, tag="pm", bufs=3, name="pm")
                    nc.vector.scalar_tensor_tensor(
                        out=pm,
                        in0=emx,
                        scalar=0.0,
                        in1=mask_in,
                        op0=AOT.bypass,
                        op1=AOT.mult,
                        accum_out=s_t[:, h : h + 1],
                    )
                    nc.vector.reciprocal(r_t[:, h : h + 1], s_t[:, h : h + 1])

                    # alpha = pm * (1/s), interleaved into o_t[:, :, h]
                    if scale_engine == "scalar":
                        nc.scalar.activation(
                            out=o_t[:, :, h],
                            in_=pm,
                            func=AFT.Copy,
                            scale=r_t[:, h : h + 1],
                        )
                    else:
                        eng = nc.gpsimd if scale_engine == "gpsimd" else nc.vector
                        eng.tensor_scalar(
                            out=o_t[:, :, h],
                            in0=pm,
                            scalar1=r_t[:, h : h + 1],
                            scalar2=None,
                            op0=AOT.mult,
                        )

                    # transposes of pm -> pmT (bf16), then accumulate out
                    pmts = []
                    for g in range(TJ // G):
                        tp = ps_tp.tile([P, G, P], PM_DT, tag="tp", name="tp")
                        for k in range(G):
                            jc = g * G + k
                            nc.tensor.transpose(
                                tp[:, k, :],
                                pm[:, jc * P : (jc + 1) * P],
                                ident_b if pm_bf16 else ident_f,
                            )
                        pmt = pmtp.tile([P, G, P], PM_DT, tag="pmt", name="pmt")
                        nc.vector.tensor_copy(
                            out=pmt.rearrange("p a b -> p (a b)").bitcast(I32),
                            in_=tp.rearrange("p a b -> p (a b)").bitcast(I32),
                        )
                        pmts.append(pmt)
                    agg = ps_agg.tile([P, C], F32, tag="agg", name="agg")
                    for jc in range(TJ):
                        nc.tensor.matmul(
                            agg,
                            lhsT=pmts[jc // G][:, jc % G, :],
                            rhs=xh_bf[:, jc, h * C : (h + 1) * C],
                            start=(jc == 0),
                            stop=(jc == TJ - 1),
                        )
                    # out = agg * (1/s) + bias
                    nc.vector.scalar_tensor_tensor(
                        out=out_sb[:, h * C : (h + 1) * C],
                        in0=agg,
                        scalar=r_t[:, h : h + 1],
                        in1=biasb_sb[:, h * C : (h + 1) * C],
                        op0=AOT.mult,
                        op1=AOT.add,
                    )

                nc.sync.dma_start(
                    out=alpha_o[t * P : (t + 1) * P], in_=o_t
                )
                nc.sync.dma_start(out=out_o[t * P : (t + 1) * P], in_=out_sb)

    if split_waits:
        _split_multi_waits(nc)
    return nc


# ---------------------------------------------------------------------------
# Host side
# ---------------------------------------------------------------------------

_NC_CACHE = {}
_NC_LOCK = threading.Lock()


def _get_nc(**kw):
    key = tuple(sorted(kw.items()))
    with _NC_LOCK:
        if key not in _NC_CACHE:
            _NC_CACHE[key] = build_gat_nc(**kw)
        return _NC_CACHE[key]


def _host_prep(x, adj, W, att_src, att_dst, bias):
    x = np.ascontiguousarray(np.asarray(x, dtype=np.float32))
    W = np.ascontiguousarray(np.asarray(W, dtype=np.float32))
    att_src = np.asarray(att_src, dtype=np.float32)
    att_dst = np.asarray(att_dst, dtype=np.float32)
    bias = np.asarray(bias, dtype=np.float32)

    attbd = np.zeros((HC, H), np.float32)
    for h in range(H):
        attbd[h * C : (h + 1) * C, h] = att_src[h]
    attdb = np.ascontiguousarray(
        np.broadcast_to(att_dst.reshape(1, HC), (P, HC))
    ).astype(np.float32)
    biasb = np.ascontiguousarray(np.broadcast_to(bias[None, :], (P, HC)))

    in_maps = []
    diag = np.arange(NI)
    for core in range(NCORES):
        b, half = core // 2, core % 2
        i0 = half * NI
        adjs = np.array(adj[b, i0 : i0 + NI, :], dtype=np.int32, copy=True)
        adjs[diag, i0 + diag] = 1
        in_maps.append(
            {
                "xb": x[b],
                "xd": np.ascontiguousarray(x[b, i0 : i0 + NI]),
                "adjs": adjs,
                "win": W,
                "attbd": attbd,
                "attdb": attdb,
                "biasb": biasb,
            }
        )
    return in_maps


def _assemble(results):
    alpha = np.empty((B, N, N, H), np.float32)
    out = np.empty((B, N, HC), np.float32)
    for core in range(NCORES):
        b, half = core // 2, core % 2
        i0 = half * NI
        alpha[b, i0 : i0 + NI] = results[core]["alpha_o"]
        out[b, i0 : i0 + NI] = results[core]["out_o"]
    return out, alpha


def run_on_hw(x, adj, W, att_src, att_dst, bias, trace=False, **build_kw):
    from concourse.bass_utils import run_bass_kernel_spmd

    nc = _get_nc(**build_kw)
    in_maps = _host_prep(x, adj, W, att_src, att_dst, bias)
    res = run_bass_kernel_spmd(
        nc, in_maps, core_ids=list(range(NCORES)), trace=trace
    )
    out, alpha = _assemble(res.results)
    return (out, alpha), res


def kernel(x, adj, W, att_src, att_dst, bias):
    (out, alpha), _ = run_on_hw(x, adj, W, att_src, att_dst, bias, trace=False)
    return out, alpha
